# revision 28
# baseline (speedup 1.0000x reference)
"""AuxSpatialGather (per-class masked mean pooling) Trainium2 kernel.

Computes, per sample b:  ctx[k, c] = mean over pixels n with gt[n]==k of feats[c, n]
(classes with zero pixels get 0), returned as [B, C, K, 1] float32.

Strategy (8 NeuronCores, data-parallel over batch, 2 samples/core):
  - feats arrive channel-major [C, HW]. They are DVE-cast fp32->fp16 right
    behind the HBM loads and PE-transposed as PAIRS of fp16 pixels viewed
    as one f32 element (halves the transpose instruction count; PE
    transpose-mode is a bit-exact raw mover), then reduced by a one-hot
    matmul in fp16 (two parity-split matmuls over a stride-2 rhs view)
    with fp32 PSUM accumulation. Per pixel group (128 px) the PE streams
    2x128 transpose cols + 512 acc cols ~ 466ns, leaving real slack under
    the ~656ns/group HBM pace - the engines never backpressure the DMA
    ring (which starves through stage-slot WAR waits when they do).
  - transpose windows are CONTIGUOUS 128-pair (256px) column blocks, so
    each window only depends on its slice of the chunk. The interleaved
    one-hot weights this implies (G_par[p,w] = gt[256w+2p+par]) are built
    on-chip from a single contiguous gt DMA: PE-transpose gt, then
    multiply by a constant permutation matrix P_par[k,m]=d(k==(2m+par)%128)
    (just identity columns re-sliced), then two stride-2 evac copies.
  - feat DMAs (2MB, 16KB partition lines) are issued a chunk ahead on the
    SP HWDGE ring; their casts are emitted interleaved into the previous
    chunk's window loop at points matching each DMA's arrival (DVE runs in
    order - early casts would head-of-line-block the PSUM evacuations).
  - the final chunk is quarter-split (ci-major within each quarter) so the
    PE tail after the last HBM byte is ~4 windows of work; the context is
    scaled by 1/max(cnt,1) and stored as [K, C] (host transposes to [C, K]).
"""

import numpy as np

NUM_CLASSES = 19
B, C, H, W = 16, 512, 128, 128
HW = H * W
N_CORES = 8
S = B // N_CORES  # samples per core
P = 128  # partitions

_compiled = None


def _build_nc(s=S, c=C, hw=HW, qw=4096):
    from concourse import bacc, mybir
    from concourse.tile import TileContext
    from concourse.masks import make_identity

    f32 = mybir.dt.float32
    f16 = mybir.dt.float16
    i32 = mybir.dt.int32
    K = NUM_CLASSES
    n_ci = c // P  # channel tiles (4)
    n_q = hw // qw  # chunks per sample (4)
    n_j = qw // 256  # pair windows per chunk (16)
    n_w = hw // 256  # pair windows per sample (64)
    n_t = hw // P  # pixel groups per sample (128)
    n_u = 4  # DMA quarter-splits for the final chunk
    uw = qw // n_u  # columns per split (1024)

    nc = bacc.Bacc("TRN2", target_bir_lowering=False)
    feats = nc.dram_tensor("feats", [s, c, hw], f32, kind="ExternalInput")
    gt = nc.dram_tensor("gt_seg_map", [s, hw], i32, kind="ExternalInput")
    # [K, c] per sample; the host transposes to [c, K]
    out = nc.dram_tensor("out", [s, K, c], f32, kind="ExternalOutput")

    with TileContext(nc) as tc:
        with (
            tc.tile_pool(name="const", bufs=1) as const_pool,
            tc.tile_pool(name="stage", bufs=7) as stage_pool,
            tc.tile_pool(name="chunks", bufs=8) as chunk_pool,
            tc.tile_pool(name="fts", bufs=4) as fts_pool,
            tc.tile_pool(name="planes", bufs=2) as plane_pool,
            tc.tile_pool(name="small", bufs=2) as small_pool,
            tc.tile_pool(name="ftp", bufs=6, space="PSUM") as ftp_pool,
            tc.tile_pool(name="accp", bufs=1, space="PSUM") as acc_pool,
            tc.tile_pool(name="tinyp", bufs=1, space="PSUM") as tiny_pool,
        ):
            ident32 = const_pool.tile([P, P], f32)
            make_identity(nc, ident32[:])
            ident16 = const_pool.tile([P, P], f16)
            make_identity(nc, ident16[:])
            ones16 = const_pool.tile([P, 1], f16)
            nc.vector.memset(ones16[:], 1.0)
            zeros16 = const_pool.tile([P, NUM_CLASSES], f16)
            nc.vector.memset(zeros16[:], 0.0)
            # P_par[k, m] = 1 iff k == (2m+par) mod 128: identity columns
            # par, par+2, ... repeated twice (the index wraps at m=64)
            perms = []
            for par in range(2):
                pm = const_pool.tile([P, P], f16, name=f"pm{par}")
                nc.vector.tensor_copy(pm[:, 0 : P // 2], ident16[:, par::2])
                nc.vector.tensor_copy(pm[:, P // 2 : P], ident16[:, par::2])
                perms.append(pm)

            def issue_dmas(si, q, n_h=1):
                """Feat loads for chunk (si, q) on the SP HWDGE ring, split
                into n_h pieces per channel tile, ci-major within each piece.
                n_h=1: full 2MB DMAs (16KB partition lines, peak HBM rate) -
                used for most chunks. The last two chunks use n_h=2/4 so the
                end-game unlocks at half/quarter granularity. Casts (DVE) are
                emitted right behind each piece: with PSUM evacs on ACT, the
                DVE stream is pure data-gated casts and cannot head-of-line
                block anything."""
                sts = [
                    stage_pool.tile([P, qw], f32, name="st") for _ in range(n_ci)
                ]
                chs = [
                    chunk_pool.tile([P, qw], f16, name="ch") for _ in range(n_ci)
                ]
                hwid = qw // n_h
                for h in range(n_h):
                    sl = slice(h * hwid, (h + 1) * hwid)
                    for ci in range(n_ci):
                        nc.sync.dma_start(
                            out=sts[ci][:, sl],
                            in_=feats[
                                si,
                                ci * P : (ci + 1) * P,
                                q * qw + h * hwid : q * qw + (h + 1) * hwid,
                            ],
                        )
                    for ci in range(n_ci):
                        nc.vector.tensor_copy(chs[ci][:, sl], sts[ci][:, sl])
                return sts, chs

            def build_planes(si):
                """One-hot planes [p, (par k w)] f16 for sample si.
                G_par[p, w] = gt[256w + 2p + par] comes from one contiguous
                gt DMA (second HWDGE ring): PE-transpose, permute rows by
                P_par, then two stride-2 column evacs per parity."""
                gt_nat = plane_pool.tile([P, n_t], i32, name="gt_nat")
                nc.scalar.dma_start(
                    out=gt_nat[:], in_=gt[si].rearrange("(p t) -> p t", p=P)
                )
                gt_f = plane_pool.tile([P, n_t], f32, name="gt_f")
                nc.vector.tensor_copy(gt_f[:], gt_nat[:])
                gtT_ps = tiny_pool.tile([P, n_t], f32, name="gtT_ps", tag="tiny")
                nc.tensor.transpose(gtT_ps[:], gt_f[:], ident32[:])
                gtT16 = plane_pool.tile([P, n_t], f16, name="gtT16")
                nc.vector.tensor_copy(gtT16[:], gtT_ps[:])
                G = plane_pool.tile([P, 2 * n_w], f16, name="G")
                for par in range(2):
                    r_ps = tiny_pool.tile([P, n_t], f32, name="r_ps", tag="tiny")
                    nc.tensor.matmul(
                        r_ps[:], perms[par][:], gtT16[:], start=True, stop=True
                    )
                    gp = G[:, par * n_w : (par + 1) * n_w]
                    # p < 64 reads even gtT block columns, p >= 64 the odd
                    nc.vector.tensor_copy(gp[: P // 2, :], r_ps[: P // 2, 0::2])
                    nc.vector.tensor_copy(gp[P // 2 :, :], r_ps[P // 2 :, 1::2])
                planes = plane_pool.tile([P, 2 * K * n_w], f16, name="planes")
                for par in range(2):
                    for k in range(K):
                        nc.vector.tensor_scalar(
                            planes[:, (par * K + k) * n_w : (par * K + k + 1) * n_w],
                            G[:, par * n_w : (par + 1) * n_w],
                            float(k),
                            None,
                            op0=mybir.AluOpType.is_equal,
                        )
                return planes

            def build_recip(planes):
                """Per-class counts -> reciprocal [K, 1] (sums both parities)."""
                partial = small_pool.tile([P, 2 * K], f32, name="partial")
                nc.vector.tensor_reduce(
                    partial[:],
                    planes[:].rearrange("p (pk w) -> p pk w", w=n_w),
                    axis=mybir.AxisListType.X,
                    op=mybir.AluOpType.add,
                )
                partial16 = small_pool.tile([P, 2 * K], f16, name="partial16")
                nc.vector.tensor_copy(partial16[:], partial[:])
                cnt_ps = tiny_pool.tile([1, 2 * K], f32, name="cnt_ps", tag="tiny")
                nc.tensor.matmul(
                    cnt_ps[:], ones16[:], partial16[:], start=True, stop=True
                )
                cnt_sb = small_pool.tile([1, 2 * K], f32, name="cnt_sb")
                nc.vector.tensor_copy(cnt_sb[:], cnt_ps[:])
                cnt_sq = small_pool.tile([32, 32], f32, name="cnt_sq")
                nc.vector.memset(cnt_sq[:], 0.0)
                nc.vector.tensor_tensor(
                    cnt_sq[:1, :K],
                    cnt_sb[:, :K],
                    cnt_sb[:, K:],
                    op=mybir.AluOpType.add,
                )
                cnt_tr = small_pool.tile([32, 32], f32, name="cnt_tr")
                nc.vector.transpose(cnt_tr[:], cnt_sq[:])
                recip = small_pool.tile([K, 1], f32, name="recip")
                nc.vector.tensor_scalar_max(recip[:], cnt_tr[:K, :1], 1.0)
                nc.vector.reciprocal(recip[:], recip[:])
                return recip

            n_chunks = s * n_q
            LAST = n_chunks - 1

            # gt+planes first (tiny DMA on the ACT ring), then feat chunk 0,
            # quarter-split so the PE starts ~15us in instead of ~30
            planes_cur = build_planes(0)
            pending = issue_dmas(0, 0, n_h=n_u)

            for si in range(s):
                acc = acc_pool.tile([K, c], f32, name="acc")
                W_all = planes_cur[:].rearrange("p (par k w) -> p par w k", par=2, k=K)
                for q in range(n_q):
                    idx = si * n_q + q
                    chs = pending[1]
                    nxt = None
                    # the last two chunks load at half/quarter granularity
                    # for a short end-game; everything earlier at full 2MB
                    n_h_next = {LAST: n_u, LAST - 1: 2}.get(idx + 1, 1)
                    if q + 1 < n_q:
                        nxt = issue_dmas(si, q + 1, n_h=n_h_next)
                    elif si + 1 < s:
                        nxt = issue_dmas(si + 1, 0, n_h=n_h_next)
                        planes_next = build_planes(si + 1)
                    if q == 0:
                        recip = build_recip(planes_cur)
                    # software-pipelined: transposes for window j are emitted
                    # LAG windows ahead of the evac + 2 matmuls for window j,
                    # so the PE never stalls waiting for its own window's
                    # PSUM evacuation (it overlaps later transposes)
                    LAG = 2
                    ftps = {}
                    for j in range(n_j + LAG):
                        if j < n_j:
                            ftp = ftp_pool.tile([P, c], f32, name="ftp", tag="ftp")
                            for ci in range(n_ci):
                                nc.tensor.transpose(
                                    ftp[:, ci * P : (ci + 1) * P],
                                    chs[ci][:].bitcast(f32)[:, j * P : (j + 1) * P],
                                    ident32[:],
                                )
                            ftps[j] = ftp
                        if j >= LAG:
                            je = j - LAG
                            w = q * n_j + je
                            ftp = ftps.pop(je)
                            fts = fts_pool.tile([P, 2 * c], f16, name="fts")
                            # all evacs on ACT: DVE then only runs casts, so
                            # neither engine's in-order stream blocks the other
                            nc.scalar.copy(fts[:].bitcast(f32), ftp[:])
                            fts_pairs = fts[:].rearrange("p (c two) -> p two c", two=2)
                            for par in range(2):
                                t = 2 * w + par
                                nc.tensor.matmul(
                                    acc[:],
                                    W_all[:, par, w, :],
                                    fts_pairs[:, par, :],
                                    start=(t == 0),
                                    stop=(t == n_t - 1),
                                )
                            # HAM keep-warm filler: accumulate an exact 0.0
                            # (zero stationary) into acc every other window.
                            # The PE's per-chunk idle clump otherwise exceeds
                            # the ~3.4us HAM window and the clock gate halves
                            # the PE clock for the next 3.4us.
                            if je % 2 == 1 and 0 < w < n_w - 1:
                                nc.tensor.matmul(
                                    acc[:],
                                    zeros16[:],
                                    fts_pairs[:, 0, :],
                                    start=False,
                                    stop=False,
                                )
                    pending = nxt

                # ---- normalize + emit [K, c] ----
                final = small_pool.tile([K, c], f32, name="final")
                nc.vector.tensor_scalar(
                    final[:], acc[:], recip[:, :1], None,
                    op0=mybir.AluOpType.mult,
                )
                # mid-stream store goes SWDGE (keeps the HWDGE rings free of
                # DMAs that wait on compute); the final store rides the idle
                # ACT HWDGE ring for its ~0.6us first-byte latency
                store_eng = nc.scalar if si == s - 1 else nc.gpsimd
                store_eng.dma_start(out=out[si], in_=final[:])
                if si + 1 < s:
                    planes_cur = planes_next
    nc.compile()
    return nc


def _get_compiled():
    global _compiled
    if _compiled is None:
        _compiled = _build_nc()
    return _compiled


def kernel(feats, gt_seg_map):
    from concourse.bass_utils import run_bass_kernel_spmd

    feats = np.asarray(feats, dtype=np.float32).reshape(B, C, HW)
    gt = np.asarray(gt_seg_map).astype(np.int32).reshape(B, HW)

    nc = _get_compiled()
    in_maps = []
    for i in range(N_CORES):
        in_maps.append(
            {
                "feats": feats[i * S : (i + 1) * S],
                "gt_seg_map": gt[i * S : (i + 1) * S],
            }
        )
    res = run_bass_kernel_spmd(nc, in_maps, core_ids=list(range(N_CORES)))
    parts = [res.results[i]["out"] for i in range(N_CORES)]  # each [S, K, C]
    full = np.concatenate(parts, axis=0)  # [B, K, C]
    return np.ascontiguousarray(np.transpose(full, (0, 2, 1)))[..., None].astype(
        np.float32
    )  # [B, C, K, 1]


# revision 29
# speedup vs baseline: 1.1129x; 1.1129x over previous
"""AuxSpatialGather (per-class masked mean pooling) Trainium2 kernel.

Computes, per sample b:  ctx[k, c] = mean over pixels n with gt[n]==k of feats[c, n]
(classes with zero pixels get 0), returned as [B, C, K, 1] float32.

Strategy (8 NeuronCores, data-parallel over batch, 2 samples/core):
  - feats arrive channel-major [C, HW]; the PE matmul contracts over the
    partition dim, so feats must become pixel-major on chip. fp32 matmul on
    TRN2 runs at ~1/4 rate, so feats are cast fp32->fp16 on DVE after plain
    f32 HWDGE loads (SWDGE cast-DMA measured ~3x slower per SDMA engine),
    then PE-transposed as PAIRS of fp16 pixels viewed as one f32 element
    (halves the transpose count; PE transpose-mode is a bit-exact raw mover),
    evacuated PSUM->SBUF (DVE/ACT alternating), and reduced by a one-hot
    matmul in fp16 (two parity-split matmuls over a stride-2 rhs view) with
    fp32 PSUM accumulation. Only precision loss: fp16 input quantization.
  - pair-windows use stride-n_j columns so the gt load lands in 32-element
    contiguous runs (fast DMA on the second HWDGE ring, off the feat FIFO).
  - transposes are emitted ci-major in groups of 4 windows so PE only needs
    the first channel tile of a chunk to start working on it: its idle at
    chunk boundaries stays under the ~3.4us HAM re-throttle window.
  - per-class counts via a free-dim reduce + ones-vector matmul; the final
    [19, 512] context is scaled by 1/max(cnt,1) and transposed to [512, 19].
"""

import numpy as np

NUM_CLASSES = 19
B, C, H, W = 16, 512, 128, 128
HW = H * W
N_CORES = 8
S = B // N_CORES  # samples per core
P = 128  # partitions

_compiled = None


def _build_nc(s=S, c=C, hw=HW, qw=4096):
    from concourse import bacc, mybir
    from concourse.tile import TileContext
    from concourse.masks import make_identity

    f32 = mybir.dt.float32
    f16 = mybir.dt.float16
    i32 = mybir.dt.int32
    K = NUM_CLASSES
    n_ci = c // P  # channel tiles (4)
    n_q = hw // qw  # n-chunks per sample (4)
    n_j = qw // 256  # pair-windows (256 pixels) per chunk (16)
    n_t = hw // P  # 128-pixel weight columns per sample (128)
    n_u = 4  # load quarters for the startup chunk

    nc = bacc.Bacc("TRN2", target_bir_lowering=False)
    feats = nc.dram_tensor("feats", [s, c, hw], f32, kind="ExternalInput")
    gt = nc.dram_tensor("gt_seg_map", [s, hw], i32, kind="ExternalInput")
    out = nc.dram_tensor("out", [s, c, K], f32, kind="ExternalOutput")

    with TileContext(nc) as tc:
        with (
            tc.tile_pool(name="const", bufs=1) as const_pool,
            tc.tile_pool(name="stage", bufs=4) as stage_pool,
            tc.tile_pool(name="chunks", bufs=3) as chunk_pool,
            tc.tile_pool(name="planes", bufs=2) as plane_pool,
            tc.tile_pool(name="ft", bufs=4) as ft_pool,
            tc.tile_pool(name="small", bufs=2) as small_pool,
            tc.tile_pool(name="ftp", bufs=5, space="PSUM") as ftp_pool,
            tc.tile_pool(name="accp", bufs=2, space="PSUM") as acc_pool,
            tc.tile_pool(name="tinyp", bufs=1, space="PSUM") as tiny_pool,
        ):
            ident32 = const_pool.tile([P, P], f32)
            make_identity(nc, ident32[:])
            ones16 = const_pool.tile([P, 1], f16)
            nc.vector.memset(ones16[:], 1.0)

            # Pixel order (all chunks): n = q*qw + 32*p + 2*j + par
            # -> G[p, t], t = q*32 + 2j + par: per-partition runs of 32
            # contiguous gt elements -> fast gt DMA; transpose windows are
            # stride-n_j pair columns.

            def load_chunks(si, q, split):
                """f32 loads + DVE casts for (si, q); split halves the loads
                (startup), else one 2MB DMA per channel tile."""
                chs = []
                for ci in range(n_ci):
                    st = stage_pool.tile([P, qw], f32, name="st")
                    ch = chunk_pool.tile([P, qw], f16, name=f"ch{ci}")
                    halves = 2 if split else 1
                    hw_half = qw // halves
                    for h in range(halves):
                        sl = slice(h * hw_half, (h + 1) * hw_half)
                        nc.sync.dma_start(
                            out=st[:, sl],
                            in_=feats[
                                si,
                                ci * P : (ci + 1) * P,
                                q * qw + h * hw_half : q * qw + (h + 1) * hw_half,
                            ],
                        )
                        nc.vector.tensor_copy(ch[:, sl], st[:, sl])
                    chs.append(ch)
                return chs

            def build_planes(si):
                """One-hot planes for sample si (pair-order pixel layout)."""
                G_i = plane_pool.tile([P, n_t], i32, name="G_i")
                # second HWDGE ring (ACT): off the FIFO ring feeding feat loads
                nc.scalar.dma_start(
                    out=G_i[:].rearrange("p (q r) -> p q r", q=n_q),
                    in_=gt[si].rearrange("(q p r) -> p q r", q=n_q, p=P),
                )
                G_f = plane_pool.tile([P, n_t], f16, name="G_f")
                nc.vector.tensor_copy(G_f[:], G_i[:])
                planes = plane_pool.tile([P, K * n_t], f16, name="planes")
                for k in range(K):
                    nc.vector.tensor_scalar(
                        planes[:, k * n_t : (k + 1) * n_t],
                        G_f[:],
                        float(k),
                        None,
                        op0=mybir.AluOpType.is_equal,
                    )
                return planes

            def build_recip(planes):
                """Per-class counts -> reciprocal [K, 1]."""
                partial = small_pool.tile([P, K], f32, name="partial")
                nc.vector.tensor_reduce(
                    partial[:],
                    planes[:].rearrange("p (k t) -> p k t", k=K),
                    axis=mybir.AxisListType.X,
                    op=mybir.AluOpType.add,
                )
                partial16 = small_pool.tile([P, K], f16, name="partial16")
                nc.vector.tensor_copy(partial16[:], partial[:])
                cnt_ps = tiny_pool.tile([1, K], f32, name="cnt_ps", tag="tiny")
                nc.tensor.matmul(
                    cnt_ps[:], ones16[:], partial16[:], start=True, stop=True
                )
                cnt_sq = small_pool.tile([32, 32], f32, name="cnt_sq")
                nc.vector.memset(cnt_sq[:], 0.0)
                nc.vector.tensor_copy(cnt_sq[:1, :K], cnt_ps[:])
                cnt_tr = small_pool.tile([32, 32], f32, name="cnt_tr")
                nc.vector.transpose(cnt_tr[:], cnt_sq[:])
                recip = small_pool.tile([K, 1], f32, name="recip")
                nc.vector.tensor_scalar_max(recip[:], cnt_tr[:K, :1], 1.0)
                nc.vector.reciprocal(recip[:], recip[:])
                return recip

            # gt+planes first (tiny DMA on the ACT ring), then the first
            # quartered chunk so the first window is ready after ~0.5MB/ci
            planes_cur = build_planes(0)
            pending = load_chunks(0, 0, split=True)

            # ---- main loop: load -> cast -> pair-transpose -> matmul ----
            for si in range(s):
                acc = acc_pool.tile([K, c], f32, name="acc")
                W_all = planes_cur[:].rearrange("p (k t) -> p t k", t=n_t)
                for q in range(n_q):
                    chs = pending
                    if q + 1 < n_q:
                        pending = load_chunks(si, q + 1, split=False)
                    elif si + 1 < s:
                        pending = load_chunks(si + 1, 0, split=False)
                        planes_next = build_planes(si + 1)
                    if q == 0:
                        recip = build_recip(planes_cur)
                    for g in range(n_j // 4):
                        # ci-major transposes within a group of 4 windows:
                        # PE needs only chunk ci0 to start this group, so its
                        # idle at chunk boundaries is spread into slivers that
                        # never trip the HAM re-throttle window.
                        ftps = [
                            ftp_pool.tile([P, c], f32, name=f"ftp{jj}", tag="ftp")
                            for jj in range(4)
                        ]
                        for ci in range(n_ci):
                            for jj in range(4):
                                j = g * 4 + jj
                                nc.tensor.transpose(
                                    ftps[jj][:, ci * P : (ci + 1) * P],
                                    chs[ci][:].bitcast(f32)[
                                        :, j : j + (P - 1) * n_j + 1 : n_j
                                    ],
                                    ident32[:],
                                )
                        for jj in range(4):
                            j = g * 4 + jj
                            fts = ft_pool.tile([P, 2 * c], f16, name="fts")
                            if j % 2 == 0:
                                nc.vector.tensor_copy(fts[:].bitcast(f32), ftps[jj][:])
                            else:
                                nc.scalar.copy(fts[:].bitcast(f32), ftps[jj][:])
                            fts_pairs = fts[:].rearrange("p (c two) -> p two c", two=2)
                            for par in range(2):
                                t = q * (n_j * 2) + 2 * j + par
                                nc.tensor.matmul(
                                    acc[:],
                                    W_all[:, t, :],
                                    fts_pairs[:, par, :],
                                    start=(t == 0),
                                    stop=(t == n_t - 1),
                                )

                # ---- normalize + emit [c, K] ----
                final = small_pool.tile([K, c], f32, name="final")
                nc.vector.tensor_scalar(
                    final[:], acc[:], recip[:, :1], None,
                    op0=mybir.AluOpType.mult,
                )
                outT_ps = tiny_pool.tile([P, n_ci * K], f32, name="outT_ps", tag="tiny")
                for ci in range(n_ci):
                    nc.tensor.transpose(
                        outT_ps[:, ci * K : (ci + 1) * K],
                        final[:K, ci * P : (ci + 1) * P],
                        ident32[:K, :K],
                    )
                outT = small_pool.tile([P, n_ci * K], f32, name="outT")
                nc.vector.tensor_copy(outT[:], outT_ps[:])
                # SWDGE: keep the HWDGE feat-load queue free of DMAs that
                # wait on compute (FIFO per issuing engine)
                nc.gpsimd.dma_start(
                    out=out[si].rearrange("(ci p) k -> p ci k", p=P),
                    in_=outT[:].rearrange("p (ci k) -> p ci k", k=K),
                )
                if si + 1 < s:
                    planes_cur = planes_next
    nc.compile()
    return nc


def _get_compiled():
    global _compiled
    if _compiled is None:
        _compiled = _build_nc()
    return _compiled


def kernel(feats, gt_seg_map):
    from concourse.bass_utils import run_bass_kernel_spmd

    feats = np.asarray(feats, dtype=np.float32).reshape(B, C, HW)
    gt = np.asarray(gt_seg_map).astype(np.int32).reshape(B, HW)

    nc = _get_compiled()
    in_maps = []
    for i in range(N_CORES):
        in_maps.append(
            {
                "feats": feats[i * S : (i + 1) * S],
                "gt_seg_map": gt[i * S : (i + 1) * S],
            }
        )
    res = run_bass_kernel_spmd(nc, in_maps, core_ids=list(range(N_CORES)))
    parts = [res.results[i]["out"] for i in range(N_CORES)]  # each [S, C, K]
    full = np.concatenate(parts, axis=0)  # [B, C, K]
    return full[..., None].astype(np.float32)  # [B, C, K, 1]

